# revision 1
# baseline (speedup 1.0000x reference)
"""Trainium2 Bass kernel for single-head causal attention with dropout.

reference:
    q,k,v = x@Wq, x@Wk, x@Wv          [B,T,H]
    wei = softmax(mask(q@k^T * H**-0.5))  (causal)
    wei = wei * (drop_u >= 0.2)/0.8
    out = wei @ v                      [B,T,H]

B=16, T=2048, D=1024, H=64. 8 NeuronCores, data-parallel over batch
(2 batches/core). Matmuls run in float32r (TF32); softmax denominator is
fused into the ScalarE exp (accum_out); dropout is a single fused
VectorE scalar_tensor_tensor op; P^T for the second matmul comes from
PE-mode transposes.
"""

import numpy as np
from contextlib import ExitStack

import concourse.bass as bass
import concourse.tile as tile
from concourse import mybir
from concourse.bass_utils import run_bass_kernel_spmd
from concourse.masks import make_identity, make_causal_mask

F32 = mybir.dt.float32
F32R = mybir.dt.float32r
BF16 = mybir.dt.bfloat16

B, T, D, H = 16, 2048, 1024, 64
N_CORES = 8
BPC = B // N_CORES          # batches per core
P_DROP = 0.2
NB = T // 128               # 16 query blocks per batch
GROUP = 4                   # query blocks per P@v group

# ---------------------------------------------------------------------------
# walrus here allows only ONE sync-wait command per instruction; Tile can
# attach several (e.g. its exit drain). Move extras onto same-engine NOPs.
def _split_excess_waits(nc):
    n = 0
    for f in nc.m.functions:
        for bb in f.blocks:
            new_insts = []
            changed = False
            for inst in bb.instructions:
                si = inst.sync_info
                if si is not None and si.on_wait and len(si.on_wait) > 1:
                    waits = list(si.on_wait)
                    extra, keep = waits[:-1], waits[-1:]
                    for i, w in enumerate(extra):
                        new_insts.append(mybir.InstNoOp(
                            name=f"{inst.name}-ws-{i}",
                            engine=inst.engine, ins=[], outs=[],
                            sync_info=mybir.SyncInfo(on_wait=[w], on_update=[]),
                            text_hint="waitsplit", bass_nofuse=True))
                        n += 1
                    si.on_wait = keep
                    changed = True
                new_insts.append(inst)
            if changed:
                bb.instructions[:] = new_insts
    return n


def _build(ctx: ExitStack, tc: "tile.TileContext", xt, wq, wk, wv, u, out):
    nc = tc.nc
    AF = mybir.ActivationFunctionType
    OP = mybir.AluOpType

    cpool = ctx.enter_context(tc.tile_pool(name="const", bufs=1))
    xpool = ctx.enter_context(tc.tile_pool(name="xt", bufs=1))
    qkvpool = ctx.enter_context(tc.tile_pool(name="qkv", bufs=2))
    vtpool = ctx.enter_context(tc.tile_pool(name="vt", bufs=1))
    epool = ctx.enter_context(tc.tile_pool(name="e", bufs=2))
    pppool = ctx.enter_context(tc.tile_pool(name="pp", bufs=2))
    upool = ctx.enter_context(tc.tile_pool(name="u", bufs=3))
    ptpool = ctx.enter_context(tc.tile_pool(name="pt", bufs=1))
    otsbpool = ctx.enter_context(tc.tile_pool(name="otsb", bufs=2))
    outpool = ctx.enter_context(tc.tile_pool(name="outsb", bufs=4))
    rspool = ctx.enter_context(tc.tile_pool(name="rs", bufs=6))
    dpool = ctx.enter_context(tc.tile_pool(name="dn", bufs=4))

    projps = ctx.enter_context(tc.tile_pool(name="projps", bufs=2, space="PSUM"))
    spsum = ctx.enter_context(tc.tile_pool(name="spsum", bufs=1, space="PSUM"))
    stageps = ctx.enter_context(tc.tile_pool(name="stage", bufs=2, space="PSUM"))
    otps = ctx.enter_context(tc.tile_pool(name="otps", bufs=1, space="PSUM"))
    onps = ctx.enter_context(tc.tile_pool(name="onps", bufs=1, space="PSUM"))

    # ---- constants -------------------------------------------------------
    ident_f = cpool.tile([128, 128], F32)
    make_identity(nc, ident_f[:])
    ident_r = cpool.tile([128, 128], F32R)
    nc.vector.tensor_copy(ident_r[:], ident_f[:])
    identb = cpool.tile([128, 128], BF16)
    make_identity(nc, identb[:])
    cmask = cpool.tile([128, 128], BF16)
    make_causal_mask(nc, cmask[:], mask_val=-1e10)

    w_sb = {}
    for name, dram in (("q", wq), ("k", wk), ("v", wv)):
        wt = cpool.tile([128, 8 * H], F32R, tag=f"w{name}")
        nc.sync.dma_start(
            wt[:].rearrange("p (c h) -> p c h", c=8),
            dram.rearrange("(c p) h -> p c h", p=128))
        w_sb[name] = wt

    for b in range(BPC):
        # ---- phase A: projections qT/kT [64,T], v [s,H] ------------------
        qT = qkvpool.tile([64, T], F32R, tag="qT")
        kT = qkvpool.tile([64, T], F32R, tag="kT")
        vT = vtpool.tile([64, T], F32R, tag="vT")
        v_sb = qkvpool.tile([128, NB * H], F32R, tag="v")

        for half in range(2):
            xts = []
            for c in range(8):
                xt_c = xpool.tile([128, T // 2], F32R, tag=f"xt{c}")
                nc.sync.dma_start(
                    xt_c[:], xt[b, 128 * c:128 * (c + 1),
                                1024 * half:1024 * (half + 1)])
                xts.append(xt_c)
            for n in range(2):
                col = 1024 * half + 512 * n
                for name, dst in (("q", qT), ("k", kT), ("v", vT)):
                    ps = projps.tile([64, 512], F32)
                    for c in range(8):
                        nc.tensor.matmul(
                            ps[:], w_sb[name][:, H * c:H * (c + 1)],
                            xts[c][:, 512 * n:512 * (n + 1)],
                            start=(c == 0), stop=(c == 7))
                    nc.scalar.copy(dst[:, col:col + 512], ps[:])

        # v: [64,T] -> natural [s,H] tiles, 8 transposes per PSUM bank
        for m in range(2):
            stage = stageps.tile([128, 512], F32R, tag="stage")
            for tloc in range(8):
                t = 8 * m + tloc
                nc.tensor.transpose(
                    stage[:, H * tloc:H * (tloc + 1)],
                    vT[:, 128 * t:128 * (t + 1)], ident_r[:64, :64])
            nc.vector.tensor_copy(
                v_sb[:, H * 8 * m:H * 8 * (m + 1)], stage[:])

        # ---- phase B: attention ------------------------------------------
        rscales = {}
        for i in range(NB):
            W = 128 * (i + 1)
            g = i // GROUP
            if i % GROUP == 0:
                ptbuf = ptpool.tile([128, NB * 512], F32R, tag="ptbuf")

            u_t = upool.tile([128, T], F32, tag="u")
            nc.sync.dma_start(u_t[:, :W], u[b, 128 * i:128 * (i + 1), :W])

            # scores + causal mask + exp(+rowsum) in <=1024-wide halves
            E = epool.tile([128, T], F32, tag="E")
            dparts = []
            for hh in range(2):
                c0 = 1024 * hh
                if c0 >= W:
                    break
                c1 = min(W, c0 + 1024)
                sps = spsum.tile([128, 1024], F32, tag="S")
                for s0 in range(c0, c1, 512):
                    n = min(512, c1 - s0)
                    is_mask_chunk = (s0 + n == W)
                    nc.tensor.matmul(
                        sps[:, s0 - c0:s0 - c0 + n],
                        qT[:, 128 * i:128 * (i + 1)], kT[:, s0:s0 + n],
                        start=True, stop=not is_mask_chunk)
                    if is_mask_chunk:
                        nc.tensor.matmul(
                            sps[:, W - 128 - c0:W - c0], identb[:], cmask[:],
                            start=False, stop=True, skip_group_check=True)
                dh = dpool.tile([128, 1], F32, tag=f"dh{hh}")
                nc.scalar.activation(
                    E[:, c0:c1], sps[:, :c1 - c0], AF.Exp,
                    scale=float(H) ** -0.5, accum_out=dh[:])
                dparts.append(dh)
            if len(dparts) == 2:
                dsum = dpool.tile([128, 1], F32, tag="dsum")
                nc.vector.tensor_add(dsum[:], dparts[0][:], dparts[1][:])
            else:
                dsum = dparts[0]
            rs = rspool.tile([128, 1], F32, tag="rs")
            nc.vector.reciprocal(rs[:], dsum[:])
            rscales[i] = rs

            # dropout: P' = (u >= p) * E, rounded to f32r
            Pp = pppool.tile([128, T], F32R, tag="Pp")
            nc.vector.scalar_tensor_tensor(
                Pp[:, :W], u_t[:, :W], P_DROP, E[:, :W],
                op0=OP.is_ge, op1=OP.mult)

            # transpose P' into ptbuf[s-chunk t, 512q-group cols]
            qoff = 128 * (i - GROUP * g)
            for m in range(0, i + 1, 4):
                k = min(4, i + 1 - m)
                stage = stageps.tile([128, 512], F32R, tag="stage")
                for jloc in range(k):
                    j = m + jloc
                    nc.tensor.transpose(
                        stage[:, 128 * jloc:128 * (jloc + 1)],
                        Pp[:, 128 * j:128 * (j + 1)], ident_r[:])
                dst = ptbuf[:].rearrange("p (t q) -> p t q", q=512)
                nc.vector.tensor_copy(
                    dst[:, m:m + k, qoff:qoff + 128],
                    stage[:, :128 * k].rearrange("p (t q) -> p t q", q=128))

            # group end: P'^T @ v for 4 query blocks at once
            if i % GROUP == GROUP - 1:
                nchunks = i + 1
                ot = otps.tile([64, 512], F32, tag="ot")
                for t in range(nchunks):
                    qo = 128 * max(0, t - GROUP * g)
                    nc.tensor.matmul(
                        ot[:, qo:512],
                        v_sb[:, H * t:H * (t + 1)],
                        ptbuf[:, 512 * t + qo:512 * (t + 1)],
                        start=(t == 0), stop=(t == nchunks - 1))
                ot_sb = otsbpool.tile([64, 512], F32, tag="otsb")
                nc.scalar.copy(ot_sb[:], ot[:])
                for cc in range(GROUP):
                    iblk = GROUP * g + cc
                    onat = onps.tile([128, 64], F32, tag="onat")
                    nc.tensor.transpose(
                        onat[:], ot_sb[:, 128 * cc:128 * (cc + 1)],
                        ident_f[:64, :64])
                    osb = outpool.tile([128, 64], F32, tag="osb")
                    nc.vector.tensor_scalar(
                        osb[:], onat[:], rscales[iblk][:], 1.0 / (1.0 - P_DROP),
                        op0=OP.mult, op1=OP.mult)
                    nc.sync.dma_start(
                        out[b, 128 * iblk:128 * (iblk + 1), :], osb[:])


_CACHE = {}


def _get_nc():
    if "nc" not in _CACHE:
        nc = bass.Bass("TRN2", target_bir_lowering=False)
        xt = nc.dram_tensor("xt", [BPC, D, T], F32R, kind="ExternalInput")
        wq = nc.dram_tensor("wq", [D, H], F32R, kind="ExternalInput")
        wk = nc.dram_tensor("wk", [D, H], F32R, kind="ExternalInput")
        wv = nc.dram_tensor("wv", [D, H], F32R, kind="ExternalInput")
        u = nc.dram_tensor("u", [BPC, T, T], F32, kind="ExternalInput")
        out = nc.dram_tensor("out", [BPC, T, H], F32, kind="ExternalOutput")
        with tile.TileContext(nc) as tc:
            with ExitStack() as ctx:
                _build(ctx, tc, xt.ap(), wq.ap(), wk.ap(), wv.ap(),
                       u.ap(), out.ap())
        _split_excess_waits(nc)
        _CACHE["nc"] = nc
    return _CACHE["nc"]


def kernel(x, Wq, Wk, Wv, drop_u, _trace=False):
    x = np.asarray(x, dtype=np.float32)
    Wq = np.asarray(Wq, dtype=np.float32)
    Wk = np.asarray(Wk, dtype=np.float32)
    Wv = np.asarray(Wv, dtype=np.float32)
    drop_u = np.asarray(drop_u, dtype=np.float32)

    nc = _get_nc()
    xt = np.ascontiguousarray(x.transpose(0, 2, 1))  # [B, D, T]
    in_maps = []
    for c in range(N_CORES):
        lo = BPC * c
        in_maps.append({
            "xt": xt[lo:lo + BPC],
            "wq": Wq, "wk": Wk, "wv": Wv,
            "u": drop_u[lo:lo + BPC],
        })
    res = run_bass_kernel_spmd(
        nc, in_maps, core_ids=list(range(N_CORES)), trace=_trace)
    out = np.concatenate([res.results[c]["out"] for c in range(N_CORES)], axis=0)
    if _trace:
        kernel.last_exec_time_ns = res.exec_time_ns
        kernel.last_results = res
    return out


# revision 4
# speedup vs baseline: 1.2083x; 1.2083x over previous
"""Trainium2 Bass kernel for single-head causal attention with dropout.

reference:
    q,k,v = x@Wq, x@Wk, x@Wv          [B,T,H]
    wei = softmax(mask(q@k^T * H**-0.5))  (causal)
    wei = wei * (drop_u >= 0.2)/0.8
    out = wei @ v                      [B,T,H]

B=16, T=2048, D=1024, H=64. 8 NeuronCores, data-parallel over batch
(2 batches/core). Matmuls run in float32r (TF32); softmax denominator is
fused into the ScalarE exp (accum_out); dropout is a single fused
VectorE scalar_tensor_tensor op; P^T for the second matmul comes from
PE-mode transposes.
"""

import numpy as np
from contextlib import ExitStack

import concourse.bass as bass
import concourse.tile as tile
from concourse import mybir
from concourse.bass_utils import run_bass_kernel_spmd
from concourse.masks import make_identity, make_causal_mask

F32 = mybir.dt.float32
F32R = mybir.dt.float32r
BF16 = mybir.dt.bfloat16

B, T, D, H = 16, 2048, 1024, 64
N_CORES = 8
BPC = B // N_CORES          # batches per core
P_DROP = 0.2
NB = T // 128               # 16 query blocks per batch
GROUP = 4                   # query blocks per P@v group

# ---------------------------------------------------------------------------
# walrus here allows only ONE sync-wait command per instruction; Tile can
# attach several (e.g. its exit drain). Move extras onto same-engine NOPs.
def _split_excess_waits(nc):
    n = 0
    for f in nc.m.functions:
        for bb in f.blocks:
            new_insts = []
            changed = False
            for inst in bb.instructions:
                si = inst.sync_info
                if si is not None and si.on_wait and len(si.on_wait) > 1:
                    waits = list(si.on_wait)
                    extra, keep = waits[:-1], waits[-1:]
                    for i, w in enumerate(extra):
                        new_insts.append(mybir.InstNoOp(
                            name=f"{inst.name}-ws-{i}",
                            engine=inst.engine, ins=[], outs=[],
                            sync_info=mybir.SyncInfo(on_wait=[w], on_update=[]),
                            text_hint="waitsplit", bass_nofuse=True))
                        n += 1
                    si.on_wait = keep
                    changed = True
                new_insts.append(inst)
            if changed:
                bb.instructions[:] = new_insts
    return n


def _build(ctx: ExitStack, tc: "tile.TileContext", xt, wq, wk, wv, u, out):
    nc = tc.nc
    AF = mybir.ActivationFunctionType
    OP = mybir.AluOpType

    cpool = ctx.enter_context(tc.tile_pool(name="const", bufs=1))
    xpool = ctx.enter_context(tc.tile_pool(name="xt", bufs=2))
    qkvpool = ctx.enter_context(tc.tile_pool(name="qkv", bufs=2))
    vtpool = ctx.enter_context(tc.tile_pool(name="vt", bufs=1))
    epool = ctx.enter_context(tc.tile_pool(name="e", bufs=2))
    pppool = ctx.enter_context(tc.tile_pool(name="pp", bufs=2))
    upool = ctx.enter_context(tc.tile_pool(name="u", bufs=3))
    ptpool = ctx.enter_context(tc.tile_pool(name="pt", bufs=1))
    otsbpool = ctx.enter_context(tc.tile_pool(name="otsb", bufs=2))
    outpool = ctx.enter_context(tc.tile_pool(name="outsb", bufs=3))
    rspool = ctx.enter_context(tc.tile_pool(name="rs", bufs=6))
    dpool = ctx.enter_context(tc.tile_pool(name="dn", bufs=4))

    projps = ctx.enter_context(tc.tile_pool(name="projps", bufs=1, space="PSUM"))
    spsum = ctx.enter_context(tc.tile_pool(name="spsum", bufs=2, space="PSUM"))
    stageps = ctx.enter_context(tc.tile_pool(name="stage", bufs=2, space="PSUM"))
    otps = ctx.enter_context(tc.tile_pool(name="otps", bufs=1, space="PSUM"))

    # ---- constants -------------------------------------------------------
    ident_f = cpool.tile([128, 128], F32)
    make_identity(nc, ident_f[:])
    ident_r = cpool.tile([128, 128], F32R)
    nc.vector.tensor_copy(ident_r[:], ident_f[:])
    identb = cpool.tile([128, 128], BF16)
    make_identity(nc, identb[:])
    cmask = cpool.tile([128, 128], BF16)
    make_causal_mask(nc, cmask[:], mask_val=-1e10)

    w_sb = {}
    for name, dram in (("q", wq), ("k", wk), ("v", wv)):
        wt = cpool.tile([128, 8 * H], F32R, tag=f"w{name}")
        nc.sync.dma_start(
            wt[:].rearrange("p (c h) -> p c h", c=8),
            dram.rearrange("(c p) h -> p c h", p=128))
        w_sb[name] = wt

    for b in range(BPC):
        # ---- phase A: projections qT/kT [64,T], v [s,H] ------------------
        qT = qkvpool.tile([64, T], F32R, tag="qT")
        kT = qkvpool.tile([64, T], F32R, tag="kT")
        vT = vtpool.tile([64, T], F32R, tag="vT")
        v_sb = qkvpool.tile([128, NB * H], F32R, tag="v")

        for quarter in range(4):
            col = 512 * quarter
            xts = []
            for c in range(8):
                xt_c = xpool.tile([128, 512], F32R, tag=f"xt{c}")
                nc.sync.dma_start(
                    xt_c[:], xt[b, 128 * c:128 * (c + 1), col:col + 512])
                xts.append(xt_c)
            for name, dst in (("q", qT), ("k", kT), ("v", vT)):
                ps = projps.tile([64, 512], F32)
                for c in range(8):
                    nc.tensor.matmul(
                        ps[:], w_sb[name][:, H * c:H * (c + 1)], xts[c][:],
                        start=(c == 0), stop=(c == 7))
                nc.scalar.copy(dst[:, col:col + 512], ps[:])

        # v: [64,T] -> natural [s,H] tiles, 8 transposes per PSUM bank
        for m in range(2):
            stage = stageps.tile([128, 512], F32R, tag="stage")
            for tloc in range(8):
                t = 8 * m + tloc
                nc.tensor.transpose(
                    stage[:, H * tloc:H * (tloc + 1)],
                    vT[:, 128 * t:128 * (t + 1)], ident_r[:64, :64])
            nc.vector.tensor_copy(
                v_sb[:, H * 8 * m:H * 8 * (m + 1)], stage[:])

        # ---- phase B: attention ------------------------------------------
        rscales = {}
        for i in range(NB):
            W = 128 * (i + 1)
            g = i // GROUP
            if i % GROUP == 0:
                ptbuf = ptpool.tile([128, NB * 512], F32R, tag="ptbuf")

            u_t = upool.tile([128, T], F32, tag="u")
            nc.sync.dma_start(u_t[:, :W], u[b, 128 * i:128 * (i + 1), :W])

            # scores + causal mask + exp(+rowsum) in <=1024-wide halves
            E = epool.tile([128, T], F32, tag="E")
            dparts = []
            for hh in range(2):
                c0 = 1024 * hh
                if c0 >= W:
                    break
                c1 = min(W, c0 + 1024)
                sps = spsum.tile([128, 1024], F32, tag="S")
                for s0 in range(c0, c1, 512):
                    n = min(512, c1 - s0)
                    is_mask_chunk = (s0 + n == W)
                    nc.tensor.matmul(
                        sps[:, s0 - c0:s0 - c0 + n],
                        qT[:, 128 * i:128 * (i + 1)], kT[:, s0:s0 + n],
                        start=True, stop=not is_mask_chunk)
                    if is_mask_chunk:
                        nc.tensor.matmul(
                            sps[:, W - 128 - c0:W - c0], identb[:], cmask[:],
                            start=False, stop=True, skip_group_check=True)
                dh = dpool.tile([128, 1], F32, tag=f"dh{hh}")
                nc.scalar.activation(
                    E[:, c0:c1], sps[:, :c1 - c0], AF.Exp,
                    scale=float(H) ** -0.5, accum_out=dh[:])
                dparts.append(dh)
            if len(dparts) == 2:
                dsum = dpool.tile([128, 1], F32, tag="dsum")
                nc.vector.tensor_add(dsum[:], dparts[0][:], dparts[1][:])
            else:
                dsum = dparts[0]
            rs = rspool.tile([128, 1], F32, tag="rs")
            nc.vector.reciprocal(rs[:], dsum[:])
            rscales[i] = rs

            # dropout: P' = (u >= p) * E, rounded to f32r
            Pp = pppool.tile([128, T], F32R, tag="Pp")
            nc.vector.scalar_tensor_tensor(
                Pp[:, :W], u_t[:, :W], P_DROP, E[:, :W],
                op0=OP.is_ge, op1=OP.mult)

            # transpose P' into ptbuf[s-chunk t, 512q-group cols]
            qoff = 128 * (i - GROUP * g)
            for m in range(0, i + 1, 4):
                k = min(4, i + 1 - m)
                stage = stageps.tile([128, 512], F32R, tag="stage")
                for jloc in range(k):
                    j = m + jloc
                    nc.tensor.transpose(
                        stage[:, 128 * jloc:128 * (jloc + 1)],
                        Pp[:, 128 * j:128 * (j + 1)], ident_r[:])
                dst = ptbuf[:].rearrange("p (t q) -> p t q", q=512)
                nc.vector.tensor_copy(
                    dst[:, m:m + k, qoff:qoff + 128],
                    stage[:, :128 * k].rearrange("p (t q) -> p t q", q=128))

            # group end: P'^T @ v for 4 query blocks at once
            if i % GROUP == GROUP - 1:
                nchunks = i + 1
                ot = otps.tile([64, 512], F32, tag="ot")
                for t in range(nchunks):
                    qo = 128 * max(0, t - GROUP * g)
                    nc.tensor.matmul(
                        ot[:, qo:512],
                        v_sb[:, H * t:H * (t + 1)],
                        ptbuf[:, 512 * t + qo:512 * (t + 1)],
                        start=(t == 0), stop=(t == nchunks - 1))
                ot_sb = otsbpool.tile([64, 512], F32, tag="otsb")
                nc.scalar.copy(ot_sb[:], ot[:])
                onat = stageps.tile([128, 256], F32, tag="stage")
                osb = outpool.tile([128, 256], F32, tag="osb")
                for cc in range(GROUP):
                    iblk = GROUP * g + cc
                    nc.tensor.transpose(
                        onat[:, 64 * cc:64 * (cc + 1)],
                        ot_sb[:, 128 * cc:128 * (cc + 1)],
                        ident_f[:64, :64])
                    nc.vector.tensor_scalar(
                        osb[:, 64 * cc:64 * (cc + 1)],
                        onat[:, 64 * cc:64 * (cc + 1)],
                        rscales[iblk][:], 1.0 / (1.0 - P_DROP),
                        op0=OP.mult, op1=OP.mult)
                nc.sync.dma_start(
                    out[b].rearrange("(c p) h -> p c h", p=128)
                       [:, GROUP * g:GROUP * (g + 1), :],
                    osb[:].rearrange("p (c h) -> p c h", c=GROUP))


_CACHE = {}


def _get_nc():
    if "nc" not in _CACHE:
        nc = bass.Bass("TRN2", target_bir_lowering=False)
        xt = nc.dram_tensor("xt", [BPC, D, T], F32R, kind="ExternalInput")
        wq = nc.dram_tensor("wq", [D, H], F32R, kind="ExternalInput")
        wk = nc.dram_tensor("wk", [D, H], F32R, kind="ExternalInput")
        wv = nc.dram_tensor("wv", [D, H], F32R, kind="ExternalInput")
        u = nc.dram_tensor("u", [BPC, T, T], F32, kind="ExternalInput")
        out = nc.dram_tensor("out", [BPC, T, H], F32, kind="ExternalOutput")
        with tile.TileContext(nc) as tc:
            with ExitStack() as ctx:
                _build(ctx, tc, xt.ap(), wq.ap(), wk.ap(), wv.ap(),
                       u.ap(), out.ap())
        _split_excess_waits(nc)
        _CACHE["nc"] = nc
    return _CACHE["nc"]


def kernel(x, Wq, Wk, Wv, drop_u, _trace=False):
    x = np.asarray(x, dtype=np.float32)
    Wq = np.asarray(Wq, dtype=np.float32)
    Wk = np.asarray(Wk, dtype=np.float32)
    Wv = np.asarray(Wv, dtype=np.float32)
    drop_u = np.asarray(drop_u, dtype=np.float32)

    nc = _get_nc()
    xt = np.ascontiguousarray(x.transpose(0, 2, 1))  # [B, D, T]
    in_maps = []
    for c in range(N_CORES):
        lo = BPC * c
        in_maps.append({
            "xt": xt[lo:lo + BPC],
            "wq": Wq, "wk": Wk, "wv": Wv,
            "u": drop_u[lo:lo + BPC],
        })
    res = run_bass_kernel_spmd(
        nc, in_maps, core_ids=list(range(N_CORES)), trace=_trace)
    out = np.concatenate([res.results[c]["out"] for c in range(N_CORES)], axis=0)
    if _trace:
        kernel.last_exec_time_ns = res.exec_time_ns
        kernel.last_results = res
    return out
